# revision 9
# baseline (speedup 1.0000x reference)
"""GAT (graph attention) layer on 8 Trainium2 NeuronCores.

Reference math (per head h):
    Wh = x @ W[h];  f_src = Wh @ a_src[h];  f_dst = Wh @ a_dst[h]
    e[i,j] = leaky_relu(f_src[i] + f_dst[j], alpha)
    att = softmax(where(adj>0, e, -9e15), axis=j)
    out[:, h*D:(h+1)*D] = att @ Wh

Exact identity used (p_i = exp(alpha*f_src_i) cancels in the softmax):
    exp(leaky_relu(s)) = exp(alpha*s) * max(1, exp((1-alpha)*s))
    nhat[j,i] = adj[j,i] * max(q_j, u_i * v_j)
      with u_i = exp((1-alpha)*f_src_i), v_j = exp(f_dst_j),
           q_j = exp(alpha*f_dst_j)
    out_h[i,:] = (sum_j nhat[j,i]*Wh[j,:]) / (sum_j nhat[j,i])

Inner loop per (j-block, head): ONE dual-scalar tensor_scalar
    t = max(U_h * v_j, q_j)     (DVE 4x mode, per-partition scalars;
                                 one head's op runs on Pool instead)
then ONE 4-head-batched tensor_tensor multiply with the bf16 0/1
adjacency tile (DVE 2x mode) and PSUM matmul accumulation.

Phase A emits the own-slice f_src/U section first, keeps all psum->SBUF
staging copies on ACT, and produces q/v per chunk so phase B's per-block
dependencies resolve as early as possible.

Distribution: destination rows i sharded 1024/core; host passes adj.T
column-slices cast to bf16 (exact for a 0/1 mask) and x pre-transposed
in bf16, so no device-side transposes or converts are needed.
"""

import os
import numpy as np
from contextlib import ExitStack

import concourse.bass as bass
import concourse.tile as tile
from concourse import bacc, mybir
from concourse.bass_utils import run_bass_kernel_spmd
from concourse.masks import make_identity

N = 8192
DIN = 256
DOUT = 64
H = 4
NCORES = 8
SL = N // NCORES          # 1024 i's per core
NB = N // 128             # 64 j-blocks
SB = SL // 128            # 8 i-blocks per core
ALPHA = 0.2
W4C = H * DOUT            # 256
WAUG = H * (DOUT + 1)     # 260

f32 = mybir.dt.float32
bf16 = mybir.dt.bfloat16
EXP = mybir.ActivationFunctionType.Exp
COPY = mybir.ActivationFunctionType.Copy
MAX = mybir.AluOpType.max
MULT = mybir.AluOpType.mult

_CACHE = {}

NBLIM = int(os.environ.get("NBLIM", str(NB)))
POOLH = int(os.environ.get("POOLH", "1"))   # heads whose TS-dual runs on Pool
BBUFS = int(os.environ.get("BBUFS", "5"))


def _build_module():
    nc = bacc.Bacc("TRN2", target_bir_lowering=False, debug=False,
                   num_devices=NCORES)

    xt_d = nc.dram_tensor("xT_full", [DIN, N], bf16, kind="ExternalInput").ap()
    xst_d = nc.dram_tensor("xT_slice", [DIN, SL], bf16, kind="ExternalInput").ap()
    w_d = nc.dram_tensor("w_all", [H, DIN, DOUT], f32, kind="ExternalInput").ap()
    a_d = nc.dram_tensor("a_all", [H, 2, DOUT], f32, kind="ExternalInput").ap()
    adjT_d = nc.dram_tensor("adjT_slice", [N, SL], bf16, kind="ExternalInput").ap()
    out_d = nc.dram_tensor("out_slice", [SL, H * DOUT], f32, kind="ExternalOutput").ap()

    with tile.TileContext(nc) as tc, ExitStack() as ctx:
        # ---------------- persistent tiles ----------------
        persist = ctx.enter_context(tc.tile_pool(name="persist", bufs=1))
        what_sb = persist.tile([128, NB, WAUG], bf16)   # Wh_aug per j-block
        u_sb = persist.tile([128, H, SL], bf16)         # exp((1-a)*f_src) bcast
        fall_sb = persist.tile([128, NB, 2 * H], f32)   # [f_src(4) | f_dst(4)]
        q_sb = persist.tile([128, H, NB], f32)          # exp(alpha*f_dst)
        v_sb = persist.tile([128, H, NB], f32)          # exp(f_dst)
        ps_sb = persist.tile([DOUT + 1, H, SL], f32)    # phase C staging
        ident_sb = persist.tile([128, 128], f32)
        make_identity(nc, ident_sb[:])

        # ======================= PHASE A =======================
        with ExitStack() as actx:
            a1pool = actx.enter_context(tc.tile_pool(name="aphase1", bufs=1))

            # --- W4 and a columns ---
            w4_sb = a1pool.tile([128, 2, W4C], f32)   # [d-part, d-chunk, h*64+o]
            for h in range(H):
                nc.sync.dma_start(
                    w4_sb[:, :, h * DOUT:(h + 1) * DOUT],
                    w_d[h].rearrange("(c p) o -> p c o", p=128))
            # a vectors broadcast across partitions (DMA step-0 AP)
            a_bc = a1pool.tile([128, H, 2, DOUT], f32)
            nc.gpsimd.dma_start(
                a_bc[:],
                bass.AP(tensor=a_d.tensor, offset=a_d.offset,
                        ap=[[0, 128]] + list(a_d.ap)))

            # --- wtilde[d, (s h)] = sum_o W4[d, o]*a[o]  (DVE reduce) ---
            wf_sb = a1pool.tile([128, 2, W4C + 8], f32)  # [W4 | wt(src4,dst4)]
            nc.vector.tensor_copy(wf_sb[:, :, 0:W4C], w4_sb[:])
            wtl_sb = a1pool.tile([128, 2, 2, H], f32)
            ttr_dump = a1pool.tile([128, DOUT], f32)
            for c in range(2):
                for s in range(2):
                    for h in range(H):
                        nc.vector.scalar_tensor_tensor(
                            out=ttr_dump[:],
                            in0=w4_sb[:, c, h * DOUT:(h + 1) * DOUT],
                            scalar=1.0,
                            in1=a_bc[:, h, s, :],
                            op0=MULT, op1=MULT,
                            accum_out=wtl_sb[:, c, s, h:h + 1])
            nc.vector.tensor_copy(
                wf_sb[:, :, W4C:],
                wtl_sb[:].rearrange("p c s h -> p c (s h)"))
            wf_b = a1pool.tile([128, 2, W4C + 8], bf16)
            nc.vector.tensor_copy(wf_b[:], wf_sb[:])

            # --- own-slice f_src -> broadcast -> U  (early: unblocks B) ---
            with ExitStack() as sctx:
                fpsum = sctx.enter_context(
                    tc.tile_pool(name="apsum_f", bufs=2, space="PSUM"))
                xst_sb = a1pool.tile([128, 2, SL], bf16)
                nc.sync.dma_start(
                    xst_sb[:], xst_d.rearrange("(c p) n -> p c n", p=128))
                fs_sb = a1pool.tile([128, SB, 4], f32)
                for bi in range(SB):
                    whf_s = fpsum.tile([128, 4], f32, tag="whfs")
                    for c in range(2):
                        nc.tensor.matmul(
                            whf_s[:],
                            xst_sb[:, c, bi * 128:(bi + 1) * 128],
                            wf_b[:, c, W4C:W4C + 4],
                            start=(c == 0), stop=(c == 1))
                    nc.scalar.activation(out=fs_sb[:, bi, :], in_=whf_s[:],
                                         func=COPY)
                fs_sb2 = fs_sb[:].rearrange("p b h -> p (b h)")
                fsT = fpsum.tile([SB * 4, 128], f32, tag="fsT", bufs=1)
                nc.tensor.transpose(fsT[:], fs_sb2, ident_sb[:])
                fsT_sb = a1pool.tile([SB * 4, 128], f32)
                nc.scalar.activation(out=fsT_sb[:], in_=fsT[:], func=COPY)
                # selection matrices: sel[:, bi, h, :] is [32,128] with row
                # (bi*4+h) all ones -> sel.T @ fsT broadcasts f_src chunk bi
                # across all 128 partitions.
                sel_sb = a1pool.tile([SB * 4, SB, H, 128], f32)
                nc.gpsimd.memset(sel_sb[:], 0.0)
                # expr = -k + 4*bi + h ; fill 1.0 where expr == 0
                nc.gpsimd.affine_select(
                    out=sel_sb[:], in_=sel_sb[:],
                    compare_op=mybir.AluOpType.not_equal,
                    fill=1.0, base=0,
                    pattern=[[4, SB], [1, H], [0, 128]],
                    channel_multiplier=-1)
                for h in range(H):
                    fbp = fpsum.tile([128, SL], f32, tag="fbp", bufs=1)
                    for bi in range(SB):
                        nc.tensor.matmul(
                            fbp[:, bi * 128:(bi + 1) * 128],
                            sel_sb[:, bi, h, :], fsT_sb[:],
                            start=True, stop=True)
                    # U_h = exp((1-alpha) * f_src_i), straight from PSUM
                    nc.scalar.activation(
                        out=u_sb[:, h, :], in_=fbp[:],
                        func=EXP, scale=(1.0 - ALPHA))

            # --- xT (host-transposed, bf16) -> [Wh | f] per n-block ---
            CB = 16
            with ExitStack() as sctx:
                apool = sctx.enter_context(tc.tile_pool(name="aphase", bufs=2))
                apsum = sctx.enter_context(
                    tc.tile_pool(name="apsum_x", bufs=2, space="PSUM"))
                for cb0 in range(0, NB, CB):
                    xt_chunk = apool.tile([128, 2, CB * 128], bf16, tag="xtchunk")
                    nc.sync.dma_start(
                        xt_chunk[:],
                        xt_d.rearrange("(c p) n -> p c n", p=128)
                            [:, :, cb0 * 128:(cb0 + CB) * 128])
                    for bi in range(CB):
                        b = cb0 + bi
                        whf = apsum.tile([128, W4C + 8], f32, tag="whf")
                        for c in range(2):
                            nc.tensor.matmul(
                                whf[:],
                                xt_chunk[:, c, bi * 128:(bi + 1) * 128],
                                wf_b[:, c, :],
                                start=(c == 0), stop=(c == 1))
                        nc.scalar.activation(
                            out=what_sb[:, b, :]
                                .rearrange("p (h o) -> p h o", h=H)[:, :, 0:DOUT],
                            in_=whf[:, 0:W4C].rearrange("p (h o) -> p h o", h=H),
                            func=COPY)
                        nc.scalar.activation(
                            out=fall_sb[:, b, :], in_=whf[:, W4C:], func=COPY)
                    # q, v for this chunk: exp(alpha*f_dst), exp(f_dst)
                    for h in range(H):
                        nc.scalar.activation(
                            out=q_sb[:, h, cb0:cb0 + CB],
                            in_=fall_sb[:, cb0:cb0 + CB, H + h],
                            func=EXP, scale=ALPHA)
                        nc.scalar.activation(
                            out=v_sb[:, h, cb0:cb0 + CB],
                            in_=fall_sb[:, cb0:cb0 + CB, H + h],
                            func=EXP, scale=1.0)
            # ones columns of Wh_aug
            for h in range(H):
                nc.vector.memset(what_sb[:, :, h * (DOUT + 1) + DOUT], 1.0)

        # ======================= PHASE B =======================
        with ExitStack() as bctx:
            bpool = bctx.enter_context(tc.tile_pool(name="bphase", bufs=BBUFS))
            bpsum = bctx.enter_context(
                tc.tile_pool(name="bpsum", bufs=1, space="PSUM"))
            ps = [bpsum.tile([DOUT + 1, SL], f32, tag=f"acc{h}", name=f"acc{h}")
                  for h in range(H)]

            for jb in range(NBLIM):
                adj_b = bpool.tile([128, SL], bf16, tag="adjb")
                nc.sync.dma_start(adj_b[:], adjT_d[jb * 128:(jb + 1) * 128, :])
                t4 = bpool.tile([128, H, SL], bf16, tag="t4")
                for h in range(H):
                    eng = nc.gpsimd if h < POOLH else nc.vector
                    eng.tensor_scalar(
                        t4[:, h, :], u_sb[:, h, :],
                        v_sb[:, h, jb:jb + 1], q_sb[:, h, jb:jb + 1],
                        op0=MULT, op1=MAX)
                # batched 4-head mask multiply (adj broadcast via stride-0)
                nh4 = bpool.tile([128, H, SL], bf16, tag="nh4")
                adj_bc = bass.AP(
                    tensor=adj_b[:].tensor, offset=adj_b[:].offset,
                    ap=[list(adj_b[:].ap[0]), [0, H], [1, SL]])
                nc.vector.tensor_tensor(nh4[:], t4[:], adj_bc, op=MULT)
                for h in range(H):
                    for half in range(2):
                        nc.tensor.matmul(
                            ps[h][:, half * 512:(half + 1) * 512],
                            what_sb[:, jb, h * (DOUT + 1):(h + 1) * (DOUT + 1)],
                            nh4[:, h, half * 512:(half + 1) * 512],
                            start=(jb == 0), stop=(jb == NBLIM - 1))

            for h in range(H):
                nc.scalar.activation(out=ps_sb[:, h, :], in_=ps[h][:],
                                     func=COPY)

        # ======================= PHASE C =======================
        with ExitStack() as cctx:
            c2pool = cctx.enter_context(tc.tile_pool(name="c2", bufs=2))
            cpsum = cctx.enter_context(
                tc.tile_pool(name="cpsum", bufs=2, space="PSUM"))
            for bi in range(SB):
                o_sb = c2pool.tile([128, H * DOUT], f32, tag="osb")
                for h in range(H):
                    pst = cpsum.tile([128, DOUT + 1], f32, tag="pst")
                    nc.tensor.transpose(
                        pst[:], ps_sb[:, h, bi * 128:(bi + 1) * 128],
                        ident_sb[0:DOUT + 1, 0:DOUT + 1])
                    rec = c2pool.tile([128, 1], f32, tag="rec")
                    nc.vector.reciprocal(rec[:], pst[:, DOUT:DOUT + 1])
                    nc.vector.tensor_scalar_mul(
                        o_sb[:, h * DOUT:(h + 1) * DOUT], pst[:, 0:DOUT], rec[:])
                nc.sync.dma_start(out_d[bi * 128:(bi + 1) * 128, :], o_sb[:])

    nc.compile()
    return nc


def kernel(x, adj, W, a_src, a_dst):
    import ml_dtypes
    x = np.asarray(x, dtype=np.float32)
    adj = np.asarray(adj)
    W = np.ascontiguousarray(np.asarray(W, dtype=np.float32))
    a_all = np.ascontiguousarray(
        np.stack([np.asarray(a_src, np.float32),
                  np.asarray(a_dst, np.float32)], axis=1))  # [H, 2, DOUT]
    # bf16 cast of the 0/1 mask is exact
    adjT_bf16 = np.ascontiguousarray(adj.T).astype(ml_dtypes.bfloat16)
    xT_bf16 = np.ascontiguousarray(x.T.astype(ml_dtypes.bfloat16))

    if "nc" not in _CACHE:
        _CACHE["nc"] = _build_module()
    nc = _CACHE["nc"]

    in_maps = []
    for c in range(NCORES):
        sl = slice(c * SL, (c + 1) * SL)
        in_maps.append({
            "xT_full": xT_bf16,
            "xT_slice": np.ascontiguousarray(xT_bf16[:, sl]),
            "w_all": W,
            "a_all": a_all,
            "adjT_slice": np.ascontiguousarray(adjT_bf16[:, sl]),
        })
    res = run_bass_kernel_spmd(nc, in_maps, core_ids=list(range(NCORES)))
    out = np.concatenate([res.results[c]["out_slice"] for c in range(NCORES)],
                         axis=0)
    return out


# revision 63
# speedup vs baseline: 1.4663x; 1.4663x over previous
"""GAT (graph attention) layer on 8 Trainium2 NeuronCores.

Reference math (per head h):
    Wh = x @ W[h];  f_src = Wh @ a_src[h];  f_dst = Wh @ a_dst[h]
    e[i,j] = leaky_relu(f_src[i] + f_dst[j], alpha)
    att = softmax(where(adj>0, e, -9e15), axis=j)
    out[:, h*D:(h+1)*D] = att @ Wh

Exact identity used (p_i = exp(alpha*f_src_i) cancels in the softmax):
    exp(leaky_relu(s)) = exp(alpha*s) * max(1, exp((1-alpha)*s))
    nhat[j,i] = adj[j,i] * max(q_j, u_i * v_j)
      with u_i = exp((1-alpha)*f_src_i), v_j = exp(f_dst_j),
           q_j = exp(alpha*f_dst_j)
    out_h[i,:] = (sum_j nhat[j,i]*Wh[j,:]) / (sum_j nhat[j,i])

Inner loop per (j-block, head-pair): two dual-scalar tensor_scalar ops
    t_h = max(U_h * v_j, q_j)        (DVE 4x mode, per-partition scalars)
then the mask multiply t_h * adj is split by columns between DVE
(tensor_tensor at 2x) and Pool (tensor_tensor, runs in parallel), and
accumulated into PSUM by the usual matmuls.

Phase B runs as two head-pair sweeps of 4 PSUM banks each so sweep 1
overlaps phase A's Wh production (which needs the other banks), and
each pair's phase C overlaps the other sweep. DMA issue order keeps
the small weight transfers ahead of the bulk xT streams; q/v/U are
produced early so the sweeps start as soon as possible.

Distribution: destination rows i sharded 1024/core; host passes adj.T
column-slices cast to bf16 (exact for a 0/1 mask) and x pre-transposed
in bf16, so no device-side transposes or converts are needed.
"""

import os
import numpy as np
from contextlib import ExitStack

import concourse.bass as bass
import concourse.tile as tile
from concourse import bacc, mybir
from concourse.bass_utils import run_bass_kernel_spmd
from concourse.masks import make_identity

N = 8192
DIN = 256
DOUT = 64
H = 4
NCORES = 8
SL = N // NCORES          # 1024 i's per core
NB = N // 128             # 64 j-blocks
SB = SL // 128            # 8 i-blocks per core
ALPHA = 0.2
W4C = H * DOUT            # 256
WAUG = H * (DOUT + 1)     # 260

f32 = mybir.dt.float32
bf16 = mybir.dt.bfloat16
EXP = mybir.ActivationFunctionType.Exp
COPY = mybir.ActivationFunctionType.Copy
MAX = mybir.AluOpType.max
MULT = mybir.AluOpType.mult

_CACHE = {}

NBLIM = int(os.environ.get("NBLIM", str(NB)))
BBUFS = int(os.environ.get("BBUFS", "4"))
CUT = int(os.environ.get("CUT", "688"))     # DVE/Pool mask column split


def _build_module():
    nc = bacc.Bacc("TRN2", target_bir_lowering=False, debug=False,
                   num_devices=NCORES)

    xt_d = nc.dram_tensor("xT_full", [DIN, N], bf16, kind="ExternalInput").ap()
    xst_d = nc.dram_tensor("xT_slice", [DIN, SL], bf16, kind="ExternalInput").ap()
    wf_d = nc.dram_tensor("wf_all", [128, 2, W4C + 8], bf16,
                          kind="ExternalInput").ap()
    adjT_d = nc.dram_tensor("adjT_slice", [N, SL], bf16, kind="ExternalInput").ap()
    sel_d = nc.dram_tensor("sel_const", [SB * 4, SB * 4, 128], bf16,
                           kind="ExternalInput").ap()
    out_d = nc.dram_tensor("out_slice", [SL, H * DOUT], f32, kind="ExternalOutput").ap()

    with tile.TileContext(nc) as tc, ExitStack() as ctx:
        # ---------------- persistent tiles ----------------
        persist = ctx.enter_context(tc.tile_pool(name="persist", bufs=1))
        what_sb = persist.tile([128, NB, WAUG], bf16)   # Wh_aug per j-block
        u_sb = persist.tile([128, H, SL], bf16)         # exp((1-a)*f_src) bcast
        fall_sb = persist.tile([128, NB, 2 * H], f32)   # [f_src(4) | f_dst(4)]
        q_sb = persist.tile([128, H, NB], f32)          # exp(alpha*f_dst)
        v_sb = persist.tile([128, H, NB], f32)          # exp(f_dst)
        ps_sb = persist.tile([DOUT + 1, H, SL], f32)    # phase C staging
        ident_sb = persist.tile([128, 128], f32)
        make_identity(nc, ident_sb[:])
        bpool = ctx.enter_context(tc.tile_pool(name="bphase", bufs=BBUFS))

        # ======================= PHASE A =======================
        with ExitStack() as actx:
            a1pool = actx.enter_context(tc.tile_pool(name="aphase1", bufs=1))

            # --- fused weight matrix [W4 | wtilde] (host-prepared) ---
            wf_b = a1pool.tile([128, 2, W4C + 8], bf16)
            nc.sync.dma_start(wf_b[:], wf_d)

            # --- own-slice f_src -> broadcast -> U  (early: unblocks B) ---
            with ExitStack() as sctx:
                fpsum = sctx.enter_context(
                    tc.tile_pool(name="apsum_f", bufs=2, space="PSUM"))
                xst_sb = a1pool.tile([128, 2, SL], bf16)
                nc.sync.dma_start(
                    xst_sb[:], xst_d.rearrange("(c p) n -> p c n", p=128))
                fs_sb = a1pool.tile([128, SB, 4], f32)
                for bi in range(SB):
                    whf_s = fpsum.tile([128, 8], f32, tag="whf8e")
                    for c in range(2):
                        nc.tensor.matmul(
                            whf_s[:, 0:4],
                            xst_sb[:, c, bi * 128:(bi + 1) * 128],
                            wf_b[:, c, W4C:W4C + 4],
                            start=(c == 0), stop=(c == 1))
                    nc.vector.tensor_copy(fs_sb[:, bi, :], whf_s[:, 0:4])
                fs_sb2 = fs_sb[:].rearrange("p b h -> p (b h)")
                fsT = fpsum.tile([SB * 4, 128], f32, tag="fsT", bufs=1)
                nc.tensor.transpose(fsT[:], fs_sb2, ident_sb[:])
                fsT_sb = a1pool.tile([SB * 4, 128], bf16)
                nc.scalar.activation(out=fsT_sb[:], in_=fsT[:], func=COPY)
                # chunk-0 f_dst/q/v hoisted here: PE computes them while
                # ACT runs the U ladder below, so sweep 1 starts early.
                # q/v emitted per 8-block half so the sweeps start off the
                # first half; sel DMA rides after xt0 (needed later).
                CB = 16
                xt0_sb = a1pool.tile([128, 2, CB * 128], bf16)
                nc.sync.dma_start(
                    xt0_sb[:],
                    xt_d.rearrange("(c p) n -> p c n", p=128)[:, :, 0:CB * 128])
                sel_sb = a1pool.tile([SB * 4, SB * 4, 128], bf16)
                nc.sync.dma_start(sel_sb[:], sel_d)
                for half in range(2):
                    for bi in range(half * 8, half * 8 + 8):
                        whf8e = fpsum.tile([128, 8], f32, tag="whf8e")
                        for c in range(2):
                            nc.tensor.matmul(
                                whf8e[:],
                                xt0_sb[:, c, bi * 128:(bi + 1) * 128],
                                wf_b[:, c, W4C:],
                                start=(c == 0), stop=(c == 1))
                        nc.vector.tensor_copy(fall_sb[:, bi, :], whf8e[:])
                    lo, hi = half * 8, half * 8 + 8
                    for h in range(H):
                        nc.scalar.activation(
                            out=q_sb[:, h, lo:hi], in_=fall_sb[:, lo:hi, H + h],
                            func=EXP, scale=ALPHA)
                        nc.scalar.activation(
                            out=v_sb[:, h, lo:hi], in_=fall_sb[:, lo:hi, H + h],
                            func=EXP, scale=1.0)
                for h in range(H):
                    fbp = fpsum.tile([128, SL], f32, tag="fbp", bufs=2)
                    for bi in range(SB):
                        nc.tensor.matmul(
                            fbp[:, bi * 128:(bi + 1) * 128],
                            sel_sb[:, bi * 4 + h, :], fsT_sb[:],
                            start=True, stop=True)
                    # U_h = exp((1-alpha) * f_src_i), straight from PSUM
                    nc.scalar.activation(
                        out=u_sb[:, h, :], in_=fbp[:],
                        func=EXP, scale=(1.0 - ALPHA))

            # ones columns of Wh_aug (before any phase-B matmul reads them)
            for h in range(H):
                nc.vector.memset(what_sb[:, :, h * (DOUT + 1) + DOUT], 1.0)

            # --- phase B sweep over a head pair, TWO j-blocks ---
            # two blocks per iteration halve the fixed per-op costs: one
            # adj DMA, one DVE mask TT and one Pool mask TT cover 4 tiles.
            def b_block(bpool, ps2, jb0, h0):
                adj2 = bpool.tile([128, 2, SL], bf16, tag="adjb")
                nc.sync.dma_start(
                    adj2[:],
                    adjT_d[jb0 * 128:(jb0 + 2) * 128, :]
                        .rearrange("(b p) i -> p b i", p=128))
                t4 = bpool.tile([128, 2, 2, SL], bf16, tag="t2")
                for blk in range(2):
                    for k, h in enumerate((h0, h0 + 1)):
                        nc.vector.tensor_scalar(
                            t4[:, blk, k, :], u_sb[:, h, :],
                            v_sb[:, h, jb0 + blk:jb0 + blk + 1],
                            q_sb[:, h, jb0 + blk:jb0 + blk + 1],
                            op0=MULT, op1=MAX)
                nh4 = bpool.tile([128, 2, 2, SL], bf16, tag="nh2")
                def adj_bc(lo, hi):
                    a = adj2[:, :, lo:hi]
                    return bass.AP(tensor=a.tensor, offset=a.offset,
                                   ap=[list(a.ap[0]), list(a.ap[1]),
                                       [0, 2], [1, hi - lo]])
                nc.vector.tensor_tensor(
                    nh4[:, :, :, 0:CUT], t4[:, :, :, 0:CUT],
                    adj_bc(0, CUT), op=MULT)
                nc.gpsimd.tensor_tensor(
                    nh4[:, :, :, CUT:], t4[:, :, :, CUT:],
                    adj_bc(CUT, SL), op=MULT)
                for blk in range(2):
                    for k, h in enumerate((h0, h0 + 1)):
                        for half in range(2):
                            nc.tensor.matmul(
                                ps2[k][:, half * 512:(half + 1) * 512],
                                what_sb[:, jb0 + blk,
                                        h * (DOUT + 1):(h + 1) * (DOUT + 1)],
                                nh4[:, blk, k, half * 512:(half + 1) * 512],
                                start=(jb0 + blk == 0),
                                stop=(jb0 + blk == NBLIM - 1))

            # --- xT -> f_dst/q/v then Wh per n-block; sweep 1 lags one
            # chunk so `what` columns land before their psum matmuls ---
            CB = 16
            with ExitStack() as sctx:
                apool = sctx.enter_context(tc.tile_pool(name="aphase", bufs=2))
                apsum = sctx.enter_context(
                    tc.tile_pool(name="apsum_x", bufs=2, space="PSUM"))
                fpsum2 = sctx.enter_context(
                    tc.tile_pool(name="apsum_f2", bufs=2, space="PSUM"))
                bpsum1 = sctx.enter_context(
                    tc.tile_pool(name="bpsum1", bufs=1, space="PSUM"))
                ps01 = [bpsum1.tile([DOUT + 1, SL], f32, tag=f"acc{h}",
                                    name=f"acc{h}") for h in range(2)]
                for cb0 in range(0, NB, CB):
                    xt_chunk = apool.tile([128, 2, CB * 128], bf16, tag="xtchunk")
                    nc.sync.dma_start(
                        xt_chunk[:],
                        xt_d.rearrange("(c p) n -> p c n", p=128)
                            [:, :, cb0 * 128:(cb0 + CB) * 128])
                    # f_dst columns first (tiny matmuls) -> q, v
                    # (chunk 0 was hoisted into the U section above)
                    for bi in (range(CB) if cb0 > 0 else ()):
                        b = cb0 + bi
                        whf8 = fpsum2.tile([128, 8], f32, tag="whf8")
                        for c in range(2):
                            nc.tensor.matmul(
                                whf8[:],
                                xt_chunk[:, c, bi * 128:(bi + 1) * 128],
                                wf_b[:, c, W4C:],
                                start=(c == 0), stop=(c == 1))
                        nc.scalar.activation(out=fall_sb[:, b, :], in_=whf8[:], func=COPY)
                    for h in (range(H) if cb0 > 0 else ()):
                        nc.scalar.activation(
                            out=q_sb[:, h, cb0:cb0 + CB],
                            in_=fall_sb[:, cb0:cb0 + CB, H + h],
                            func=EXP, scale=ALPHA)
                        nc.scalar.activation(
                            out=v_sb[:, h, cb0:cb0 + CB],
                            in_=fall_sb[:, cb0:cb0 + CB, H + h],
                            func=EXP, scale=1.0)
                    # Wh columns, interleaved 1:1 with sweep-1 blocks of
                    # the PREVIOUS chunk so PE never sees a long A2 burst
                    for bi in range(CB):
                        b = cb0 + bi
                        whf = apsum.tile([128, W4C], f32, tag="whf")
                        for c in range(2):
                            nc.tensor.matmul(
                                whf[:],
                                xt_chunk[:, c, bi * 128:(bi + 1) * 128],
                                wf_b[:, c, 0:W4C],
                                start=(c == 0), stop=(c == 1))
                        nc.scalar.activation(
                            out=what_sb[:, b, :]
                                .rearrange("p (h o) -> p h o", h=H)[:, :, 0:DOUT],
                            in_=whf[:].rearrange("p (h o) -> p h o", h=H),
                            func=COPY)
                        if cb0 > 0 and bi % 2 == 0 and cb0 - CB + bi < NBLIM:
                            b_block(bpool, ps01, cb0 - CB + bi, 0)
                for jb in range(NB - CB, min(NB, NBLIM), 2):
                    b_block(bpool, ps01, jb, 0)
                for h in range(2):
                    nc.scalar.activation(out=ps_sb[:, h, :], in_=ps01[h][:],
                                         func=COPY)

        # ============ PHASE B sweep 2 + PHASE C per head pair ============
        with ExitStack() as tctx:
            c2pool = tctx.enter_context(tc.tile_pool(name="c2", bufs=4))
            cpsum = tctx.enter_context(
                tc.tile_pool(name="cpsum", bufs=4, space="PSUM"))

            def c_bi(h0, bi, o_all):
                # output rows bi*128.. for heads h0, h0+1 into pair staging
                for k, h in enumerate((h0, h0 + 1)):
                    pst = cpsum.tile([128, DOUT + 1], f32, tag="pst")
                    nc.tensor.transpose(
                        pst[:], ps_sb[:, h, bi * 128:(bi + 1) * 128],
                        ident_sb[0:DOUT + 1, 0:DOUT + 1])
                    rec = c2pool.tile([128, 1], f32, tag="rec")
                    nc.vector.reciprocal(rec[:], pst[:, DOUT:DOUT + 1])
                    nc.scalar.activation(
                        out=o_all[:, bi, k * DOUT:(k + 1) * DOUT],
                        in_=pst[:, 0:DOUT], func=COPY, scale=rec[:])

            def c_flush(h0, o_all):
                # single DMA for the whole pair: rows grouped per 128-block
                nc.sync.dma_start(
                    out_d.rearrange("(b p) c -> p b c", p=128)
                         [:, :, h0 * DOUT:(h0 + 2) * DOUT], o_all[:])

            with ExitStack() as bctx:
                bpsum2 = bctx.enter_context(
                    tc.tile_pool(name="bpsum2", bufs=1, space="PSUM"))
                ps23 = [bpsum2.tile([DOUT + 1, SL], f32, tag=f"acc{h+2}",
                                    name=f"acc{h+2}") for h in range(2)]
                o_all0 = c2pool.tile([128, SB, 2 * DOUT], f32, tag="oall0")
                for jb in range(0, NBLIM, 2):
                    b_block(bpool, ps23, jb, 2)
                    # pair-0 epilogue interleaved so its DVE/PE ops never
                    # head-block sweep 2's queue
                    if jb % 8 == 6:
                        c_bi(0, jb // 8, o_all0)
                c_flush(0, o_all0)
                for h in range(2):
                    nc.scalar.activation(out=ps_sb[:, h + 2, :],
                                         in_=ps23[h][:], func=COPY)
            o_all2 = c2pool.tile([128, SB, 2 * DOUT], f32, tag="oall2")
            for bi in range(SB):
                c_bi(2, bi, o_all2)
            c_flush(2, o_all2)

    nc.compile()
    return nc


def kernel(x, adj, W, a_src, a_dst):
    import ml_dtypes
    x = np.asarray(x, dtype=np.float32)
    adj = np.asarray(adj)
    W = np.ascontiguousarray(np.asarray(W, dtype=np.float32))
    a_all = np.ascontiguousarray(
        np.stack([np.asarray(a_src, np.float32),
                  np.asarray(a_dst, np.float32)], axis=1))  # [H, 2, DOUT]
    # fused weights: [W4 | wtilde] with wtilde = W @ a (weight-only prep)
    wt = np.einsum('hdo,hso->hds', W, a_all)          # [H, DIN, 2]
    wf = np.zeros((128, 2, W4C + 8), dtype=np.float32)
    for h in range(H):
        for c in range(2):
            wf[:, c, h * DOUT:(h + 1) * DOUT] = W[h, c * 128:(c + 1) * 128, :]
            for s in range(2):
                wf[:, c, W4C + s * 4 + h] = wt[h, c * 128:(c + 1) * 128, s]
    wf_bf16 = wf.astype(ml_dtypes.bfloat16)
    # bf16 cast of the 0/1 mask is exact
    adjT_bf16 = np.ascontiguousarray(adj.T).astype(ml_dtypes.bfloat16)
    xT_bf16 = np.ascontiguousarray(x.T.astype(ml_dtypes.bfloat16))
    # one-hot row-selection constant for the f_src broadcast matmuls
    import ml_dtypes as _mld
    sel = np.zeros((SB * 4, SB * 4, 128), dtype=_mld.bfloat16)
    for k in range(SB * 4):
        sel[k, k, :] = 1.0

    if "nc" not in _CACHE:
        _CACHE["nc"] = _build_module()
    nc = _CACHE["nc"]

    in_maps = []
    for c in range(NCORES):
        sl = slice(c * SL, (c + 1) * SL)
        in_maps.append({
            "xT_full": xT_bf16,
            "xT_slice": np.ascontiguousarray(xT_bf16[:, sl]),
            "wf_all": wf_bf16,
            "adjT_slice": np.ascontiguousarray(adjT_bf16[:, sl]),
            "sel_const": sel,
        })
    res = run_bass_kernel_spmd(nc, in_maps, core_ids=list(range(NCORES)))
    out = np.concatenate([res.results[c]["out_slice"] for c in range(NCORES)],
                         axis=0)
    return out
